# revision 11
# baseline (speedup 1.0000x reference)
"""Causal self-attention on 8 TRN2 NeuronCores.

Sharding: B=4 batches x 16 heads -> 64 (b,h) pairs; core c handles batch
b=c//2 and head-group hg=c%2 (8 heads = 512 of the 1024 features).
Q/K/V projection weights are row-sliced per head group (column-sharded in
the x @ W.T sense), so each core computes its own (b, 8-head) slice of the
S x S attention without any cross-core communication.

Kernel layout choices:
- Matmuls contract over SBUF partitions, so X^T (and W^T) are materialized
  on-chip via PE transpose-mode matmuls (fp32 has no DMA transpose). The
  X^T transposes are interleaved with the V-projection matmuls to keep the
  PE HAM clock-gate warm.
- All big matmuls run as float32r (full PE rate at free-dim >=256, ~tf32
  precision). Tiles feeding matmuls are allocated float32r so the
  producing op does the rounding (a bitcast is rejected by the verifier).
- Scores are computed transposed, S^T[k, q] = (K^T)^T Q^T per 128-key chunk,
  so that softmax(S)@V becomes out^T[d, q] = V^T P^T with a 512-wide moving
  operand. Heads are processed in pairs: head parity picks partitions 0-63
  vs 64-127 (independent PE row groups -> the two K=64 matmuls overlap).
- No row-max subtraction: scaled scores are ~N(0,1), exp is safe in fp32.
  exp runs on ScalarE straight from PSUM with the attention-mask bias and
  1/sqrt(64) scale fused in. Causal structure is exploited by narrowing
  the q-window of QK/exp/AV on diagonal tiles; only the single partial
  128-col window needs a 0/1 mask multiply after exp.
- A ones-column appended to V makes the AV matmul also produce the softmax
  denominator (row 64 of the [65, 512] PSUM accumulator).
- Output heads are PE-transposed back to [seq, d] and normalized by the
  reciprocal of the denominator column on the way out.
"""

import sys

if "/opt/trn_rl_repo" not in sys.path:
    sys.path.insert(0, "/opt/trn_rl_repo")

import numpy as np

B, S, H, NH = 4, 2048, 1024, 16
HD = 64
NCORES = 8
F = 512  # features per core (8 heads)
NHEADS = 8  # heads per core
NPAIR = 4  # head pairs per core
HCH = H // 128  # 8 hidden chunks
SCH = S // 128  # 16 sequence chunks
P = 128

_CACHE = {}


def _build_bass():
    import concourse.tile as tile
    from concourse import bacc, mybir
    from contextlib import ExitStack

    f32 = mybir.dt.float32
    f32r = mybir.dt.float32r
    EXP = mybir.ActivationFunctionType.Exp
    ADD = mybir.AluOpType.add
    MULT = mybir.AluOpType.mult

    nc = bacc.Bacc("TRN2", target_bir_lowering=False, debug=False, num_devices=NCORES)

    x_d = nc.dram_tensor("x", [S, H], f32, kind="ExternalInput").ap()
    wq_d = nc.dram_tensor("wq", [F, H], f32, kind="ExternalInput").ap()
    wk_d = nc.dram_tensor("wk", [F, H], f32, kind="ExternalInput").ap()
    wv_d = nc.dram_tensor("wv", [F, H], f32, kind="ExternalInput").ap()
    bqt_d = nc.dram_tensor("bqt", [P, NPAIR], f32, kind="ExternalInput").ap()
    bkt_d = nc.dram_tensor("bkt", [P, NPAIR], f32, kind="ExternalInput").ap()
    bvb_d = nc.dram_tensor("bvb", [P, F], f32, kind="ExternalInput").ap()
    maskb_d = nc.dram_tensor("maskb", [P, SCH], f32, kind="ExternalInput").ap()
    cm_d = nc.dram_tensor("cm", [P, P], f32, kind="ExternalInput").ap()
    id_d = nc.dram_tensor("ident", [P, P], f32, kind="ExternalInput").ap()
    out_d = nc.dram_tensor("out", [S, F], f32, kind="ExternalOutput").ap()

    with tile.TileContext(nc) as tc, ExitStack() as ctx:
        const = ctx.enter_context(tc.tile_pool(name="const", bufs=1))
        ident = const.tile([P, P], f32, tag="ident")
        nc.sync.dma_start(ident[:], id_d[:])
        cm = const.tile([P, P], f32, tag="cm")
        nc.sync.dma_start(cm[:], cm_d[:])
        maskb = const.tile([P, SCH], f32, tag="maskb")
        nc.sync.dma_start(maskb[:], maskb_d[:])
        bqt = const.tile([P, NPAIR], f32, tag="bqt")
        nc.sync.dma_start(bqt[:], bqt_d[:])
        bkt = const.tile([P, NPAIR], f32, tag="bkt")
        nc.sync.dma_start(bkt[:], bkt_d[:])
        bvb = const.tile([P, F], f32, tag="bvb")
        nc.sync.dma_start(bvb[:], bvb_d[:])

        xt_pool = ctx.enter_context(tc.tile_pool(name="xt", bufs=1))
        xt = xt_pool.tile([P, HCH, S], f32r, tag="xt")  # X^T, 64KB/partition
        v_pool = ctx.enter_context(tc.tile_pool(name="v", bufs=1))
        v = v_pool.tile([P, SCH, NHEADS, HD + 1], f32r, tag="v")  # V + ones col

        stage = ctx.enter_context(tc.tile_pool(name="stage", bufs=2))
        # One shared PSUM pool/tag for QK pairs, projections, and all PE
        # transposes: 3 slots x 2 banks. The AV accumulators get their own
        # 2 slots x 1 bank. Total 8 banks.
        mmps = ctx.enter_context(tc.tile_pool(name="mmps", bufs=3, space="PSUM"))
        ops_ = ctx.enter_context(tc.tile_pool(name="ops", bufs=2, space="PSUM"))
        wt_pool = ctx.enter_context(tc.tile_pool(name="wt", bufs=2))
        qkt_pool = ctx.enter_context(tc.tile_pool(name="qkt", bufs=2))
        p_pool = ctx.enter_context(tc.tile_pool(name="pp", bufs=4))
        ot_pool = ctx.enter_context(tc.tile_pool(name="ot", bufs=2))
        obuf = ctx.enter_context(tc.tile_pool(name="obuf", bufs=4))
        rec_pool = ctx.enter_context(tc.tile_pool(name="rec", bufs=4))

        def transpose_pair(dst_ap, src_a, src_b):
            # dst[128, 2, 128] = stack(src_a.T, src_b.T); two PE transposes
            # into the two banks of one mm slot, one strided DVE copyback.
            slot = mmps.tile([P, 1024], f32, tag="mm")
            nc.tensor.transpose(slot[:, 0:128], src_a, ident[:])
            nc.tensor.transpose(slot[:, 512:640], src_b, ident[:])
            nc.vector.tensor_copy(
                dst_ap,
                slot[:].rearrange("p (a b) -> p a b", a=2)[:, :, 0:128],
            )

        # ---- Wv^T (8 transposes per 128-row chunk of wv) ----
        with tc.tile_pool(name="wtv", bufs=1) as wtv_pool:
            wtv = wtv_pool.tile([P, HCH, F], f32r, tag="wtv")
            for rc in range(4):
                ws = stage.tile([P, H], f32, tag="stage")
                nc.sync.dma_start(ws[:], wv_d[rc * 128 : (rc + 1) * 128, :])
                for j in range(0, HCH, 2):
                    transpose_pair(
                        wtv[:, j : j + 2, rc * 128 : (rc + 1) * 128],
                        ws[:, j * 128 : (j + 1) * 128],
                        ws[:, (j + 1) * 128 : (j + 2) * 128],
                    )
            # ones column of V (tensor_scalar: in0*0 + 1; memset can't write f32r)
            nc.vector.tensor_scalar(
                v[:, :, :, HD : HD + 1],
                bvb[:, 0:128].rearrange("p (a b c) -> p a b c", a=SCH, b=NHEADS),
                0.0,
                1.0,
                MULT,
                ADD,
            )
            # ---- X^T interleaved with V = X @ Wv_s^T + bv ----
            for si in range(SCH):
                xs = stage.tile([P, H], f32, tag="stage")
                nc.sync.dma_start(xs[:], x_d[si * 128 : (si + 1) * 128, :])
                for j in range(0, HCH, 2):
                    transpose_pair(
                        xt[:, j : j + 2, si * 128 : (si + 1) * 128],
                        xs[:, j * 128 : (j + 1) * 128],
                        xs[:, (j + 1) * 128 : (j + 2) * 128],
                    )
                ps = mmps.tile([P, 1024], f32, tag="mm")
                for j in range(HCH):
                    nc.tensor.matmul(
                        ps[:, 0:F],
                        xt[:, j, si * 128 : (si + 1) * 128],
                        wtv[:, j, :],
                        start=(j == 0),
                        stop=(j == HCH - 1),
                    )
                nc.vector.tensor_tensor(
                    v[:, si, :, 0:HD],
                    ps[:, 0:F].rearrange("p (h d) -> p h d", h=NHEADS),
                    bvb[:].rearrange("p (h d) -> p h d", h=NHEADS),
                    ADD,
                )

        # ---- per head-pair: project Q^T/K^T then attention ----
        for pr in range(NPAIR):
            h0, h1 = 2 * pr, 2 * pr + 1
            wtq = wt_pool.tile([P, HCH, P], f32r, tag="wtq")
            wtk = wt_pool.tile([P, HCH, P], f32r, tag="wtk")
            for wd, wt in ((wq_d, wtq), (wk_d, wtk)):
                ws = stage.tile([P, H], f32, tag="stage")
                nc.sync.dma_start(ws[:], wd[pr * 128 : (pr + 1) * 128, :])
                for j in range(0, HCH, 2):
                    transpose_pair(
                        wt[:, j : j + 2, :],
                        ws[:, j * 128 : (j + 1) * 128],
                        ws[:, (j + 1) * 128 : (j + 2) * 128],
                    )

            qt = qkt_pool.tile([P, S], f32r, tag="qt")
            kt = qkt_pool.tile([P, S], f32r, tag="kt")
            for wt, dst, bias in ((wtq, qt, bqt), (wtk, kt, bkt)):
                for st in range(4):
                    ps = mmps.tile([P, 1024], f32, tag="mm")
                    for j in range(HCH):
                        nc.tensor.matmul(
                            ps[:, 0:F],
                            wt[:, j, :],
                            xt[:, j, st * 512 : (st + 1) * 512],
                            start=(j == 0),
                            stop=(j == HCH - 1),
                        )
                    nc.vector.tensor_scalar_add(
                        dst[:, st * 512 : (st + 1) * 512],
                        ps[:, 0:F],
                        bias[:, pr : pr + 1],
                    )

            for qi in range(4):
                q0 = qi * 512
                nk = 4 * (qi + 1)
                oa = ops_.tile([P, F], f32, tag="o")
                ob = ops_.tile([P, F], f32, tag="o")
                for kc in range(nk):
                    off = kc - 4 * qi  # >=0 on diagonal-band tiles
                    lo = off * 128 if off > 0 else 0  # valid q-window start
                    ps = mmps.tile([P, 1024], f32, tag="mm")
                    nc.tensor.matmul(
                        ps[:, lo:512],
                        kt[0:64, kc * 128 : (kc + 1) * 128],
                        qt[0:64, q0 + lo : q0 + 512],
                        start=True,
                        stop=True,
                    )
                    nc.tensor.matmul(
                        ps[:, 512 + lo : 1024],
                        kt[64:128, kc * 128 : (kc + 1) * 128],
                        qt[64:128, q0 + lo : q0 + 512],
                        start=True,
                        stop=True,
                    )
                    pt = p_pool.tile([P, 1024], f32r, tag="pt")
                    if lo == 0:
                        nc.scalar.activation(
                            pt[:], ps[:], EXP, bias=maskb[:, kc : kc + 1], scale=0.125
                        )
                    else:
                        nc.scalar.activation(
                            pt[:, lo:512],
                            ps[:, lo:512],
                            EXP,
                            bias=maskb[:, kc : kc + 1],
                            scale=0.125,
                        )
                        nc.scalar.activation(
                            pt[:, 512 + lo : 1024],
                            ps[:, 512 + lo : 1024],
                            EXP,
                            bias=maskb[:, kc : kc + 1],
                            scale=0.125,
                        )
                    if off >= 0:
                        # mask the single partial 128-col window of each head
                        nc.vector.tensor_mul(
                            pt[:, lo : lo + 128], pt[:, lo : lo + 128], cm[:]
                        )
                        nc.vector.tensor_mul(
                            pt[:, 512 + lo : 512 + lo + 128],
                            pt[:, 512 + lo : 512 + lo + 128],
                            cm[:],
                        )
                    nc.tensor.matmul(
                        oa[0 : HD + 1, lo:512],
                        v[:, kc, h0, :],
                        pt[:, lo:512],
                        start=(kc == 0),
                        stop=(kc == nk - 1),
                    )
                    nc.tensor.matmul(
                        ob[0 : HD + 1, lo:512],
                        v[:, kc, h1, :],
                        pt[:, 512 + lo : 1024],
                        start=(kc == 0),
                        stop=(kc == nk - 1),
                    )
                for o_ps, h in ((oa, h0), (ob, h1)):
                    ot = ot_pool.tile([HD + 1, F], f32, tag="ot")
                    nc.vector.tensor_copy(ot[:], o_ps[0 : HD + 1, :])
                    for t2 in range(2):  # two transposes per mm slot
                        slot = mmps.tile([P, 1024], f32, tag="mm")
                        rec = rec_pool.tile([P, 2], f32, tag="rec")
                        for u in range(2):
                            t = 2 * t2 + u
                            nc.tensor.transpose(
                                slot[:, 512 * u : 512 * u + HD + 1],
                                ot[:, t * 128 : (t + 1) * 128],
                                ident[0 : HD + 1, 0 : HD + 1],
                            )
                        nc.vector.reciprocal(
                            rec[:],
                            slot[:].rearrange("p (a b) -> p a b", a=2)[:, :, HD],
                        )
                        for u in range(2):
                            t = 2 * t2 + u
                            otile = obuf.tile([P, HD], f32, tag="ob")
                            nc.vector.tensor_scalar_mul(
                                otile[:],
                                slot[:, 512 * u : 512 * u + HD],
                                rec[:, u : u + 1],
                            )
                            nc.sync.dma_start(
                                out_d[
                                    q0 + t * 128 : q0 + (t + 1) * 128,
                                    h * HD : (h + 1) * HD,
                                ],
                                otile[:],
                            )

    nc.compile()
    return nc


def _get_nc():
    if "nc" not in _CACHE:
        _CACHE["nc"] = _build_bass()
    return _CACHE["nc"]


def _host_consts():
    if "consts" not in _CACHE:
        qq = np.arange(P)[None, :]
        kk = np.arange(P)[:, None]
        _CACHE["consts"] = {
            "cm": (qq >= kk).astype(np.float32),
            "ident": np.eye(P, dtype=np.float32),
        }
    return _CACHE["consts"]


def make_in_maps(inputs):
    hs = np.asarray(inputs["hidden_states"], dtype=np.float32)
    am = np.asarray(inputs["attention_mask"], dtype=np.float32)
    Wq = np.asarray(inputs["Wq"], dtype=np.float32)
    bq = np.asarray(inputs["bq"], dtype=np.float32)
    Wk = np.asarray(inputs["Wk"], dtype=np.float32)
    bk = np.asarray(inputs["bk"], dtype=np.float32)
    Wv = np.asarray(inputs["Wv"], dtype=np.float32)
    bv = np.asarray(inputs["bv"], dtype=np.float32)

    consts = _host_consts()
    in_maps = []
    for c in range(NCORES):
        b, hg = c // 2, c % 2
        fsl = slice(hg * F, (hg + 1) * F)
        in_maps.append(
            {
                "x": np.ascontiguousarray(hs[b]),
                "wq": np.ascontiguousarray(Wq[fsl]),
                "wk": np.ascontiguousarray(Wk[fsl]),
                "wv": np.ascontiguousarray(Wv[fsl]),
                "bqt": np.ascontiguousarray(bq[fsl].reshape(NPAIR, P).T),
                "bkt": np.ascontiguousarray(bk[fsl].reshape(NPAIR, P).T),
                "bvb": np.broadcast_to(bv[fsl], (P, F)).copy(),
                "maskb": np.ascontiguousarray((am[b, 0, 0] / 8.0).reshape(SCH, P).T),
                "cm": consts["cm"],
                "ident": consts["ident"],
            }
        )
    return in_maps


def assemble_out(results):
    out = np.empty((B, S, H), dtype=np.float32)
    for c in range(NCORES):
        b, hg = c // 2, c % 2
        out[b, :, hg * F : (hg + 1) * F] = results[c]["out"]
    return out


def kernel(**inputs):
    from concourse.bass_utils import run_bass_kernel_spmd

    in_maps = make_in_maps(inputs)
    nc = _get_nc()
    res = run_bass_kernel_spmd(nc, in_maps, list(range(NCORES)))
    return assemble_out(res.results)


if __name__ == "__main__":
    rng = np.random.default_rng(0)
    ins = {
        "hidden_states": rng.standard_normal((B, S, H)).astype(np.float32),
        "attention_mask": np.zeros((B, 1, 1, S), np.float32),
        "Wq": (rng.standard_normal((H, H)) / 32.0).astype(np.float32),
        "bq": np.zeros(H, np.float32),
        "Wk": (rng.standard_normal((H, H)) / 32.0).astype(np.float32),
        "bk": np.zeros(H, np.float32),
        "Wv": (rng.standard_normal((H, H)) / 32.0).astype(np.float32),
        "bv": np.zeros(H, np.float32),
    }
    o = kernel(**ins)
    print("out", o.shape, o.dtype, float(np.abs(o).max()))


# revision 13
# speedup vs baseline: 1.0657x; 1.0657x over previous
"""Causal self-attention on 8 TRN2 NeuronCores.

Sharding: B=4 batches x 16 heads -> 64 (b,h) pairs; core c handles batch
b=c//2 and head-group hg=c%2 (8 heads = 512 of the 1024 features).
Q/K/V projection weights are row-sliced per head group (column-sharded in
the x @ W.T sense), so each core computes its own (b, 8-head) slice of the
S x S attention without any cross-core communication.

Kernel layout choices:
- Matmuls contract over SBUF partitions, so X^T (and W^T) are materialized
  on-chip via PE transpose-mode matmuls (fp32 has no DMA transpose). The
  X^T transposes are interleaved with the V-projection matmuls to keep the
  PE HAM clock-gate warm.
- All big matmuls run as float32r (full PE rate at free-dim >=256, ~tf32
  precision). Tiles feeding matmuls are allocated float32r so the
  producing op does the rounding (a bitcast is rejected by the verifier).
- Scores are computed transposed, S^T[k, q] = (K^T)^T Q^T per 128-key chunk,
  so that softmax(S)@V becomes out^T[d, q] = V^T P^T with a 512-wide moving
  operand. Heads are processed in pairs: head parity picks partitions 0-63
  vs 64-127 (independent PE row groups -> the two K=64 matmuls overlap).
- No row-max subtraction: scaled scores are ~N(0,1), exp is safe in fp32.
  exp runs on ScalarE straight from PSUM with the attention-mask bias and
  1/sqrt(64) scale fused in. Causal structure is exploited by narrowing
  the q-window of QK/exp/AV on diagonal tiles; only the single partial
  128-col window needs a 0/1 mask multiply after exp.
- A ones-column appended to V makes the AV matmul also produce the softmax
  denominator (row 64 of the [65, 512] PSUM accumulator).
- Output heads are PE-transposed back to [seq, d] and normalized by the
  reciprocal of the denominator column on the way out.
"""

import sys

if "/opt/trn_rl_repo" not in sys.path:
    sys.path.insert(0, "/opt/trn_rl_repo")

import numpy as np

B, S, H, NH = 4, 2048, 1024, 16
HD = 64
NCORES = 8
F = 512  # features per core (8 heads)
NHEADS = 8  # heads per core
NPAIR = 4  # head pairs per core
HCH = H // 128  # 8 hidden chunks
SCH = S // 128  # 16 sequence chunks
P = 128

_CACHE = {}


def _build_bass():
    import concourse.tile as tile
    from concourse import bacc, mybir
    from contextlib import ExitStack

    f32 = mybir.dt.float32
    f32r = mybir.dt.float32r
    EXP = mybir.ActivationFunctionType.Exp
    ADD = mybir.AluOpType.add
    MULT = mybir.AluOpType.mult

    nc = bacc.Bacc("TRN2", target_bir_lowering=False, debug=False, num_devices=NCORES)

    x_d = nc.dram_tensor("x", [S, H], f32, kind="ExternalInput").ap()
    wq_d = nc.dram_tensor("wq", [F, H], f32, kind="ExternalInput").ap()
    wk_d = nc.dram_tensor("wk", [F, H], f32, kind="ExternalInput").ap()
    wv_d = nc.dram_tensor("wv", [F, H], f32, kind="ExternalInput").ap()
    bqt_d = nc.dram_tensor("bqt", [P, NPAIR], f32, kind="ExternalInput").ap()
    bkt_d = nc.dram_tensor("bkt", [P, NPAIR], f32, kind="ExternalInput").ap()
    bvb_d = nc.dram_tensor("bvb", [P, F], f32, kind="ExternalInput").ap()
    maskb_d = nc.dram_tensor("maskb", [P, SCH], f32, kind="ExternalInput").ap()
    cm_d = nc.dram_tensor("cm", [P, P], f32, kind="ExternalInput").ap()
    id_d = nc.dram_tensor("ident", [P, P], f32, kind="ExternalInput").ap()
    out_d = nc.dram_tensor("out", [S, F], f32, kind="ExternalOutput").ap()

    with tile.TileContext(nc) as tc, ExitStack() as ctx:
        const = ctx.enter_context(tc.tile_pool(name="const", bufs=1))
        ident = const.tile([P, P], f32, tag="ident")
        nc.sync.dma_start(ident[:], id_d[:])
        cm = const.tile([P, P], f32, tag="cm")
        nc.sync.dma_start(cm[:], cm_d[:])
        maskb = const.tile([P, SCH], f32, tag="maskb")
        nc.sync.dma_start(maskb[:], maskb_d[:])
        bqt = const.tile([P, NPAIR], f32, tag="bqt")
        nc.sync.dma_start(bqt[:], bqt_d[:])
        bkt = const.tile([P, NPAIR], f32, tag="bkt")
        nc.sync.dma_start(bkt[:], bkt_d[:])
        bvb = const.tile([P, F], f32, tag="bvb")
        nc.sync.dma_start(bvb[:], bvb_d[:])

        xt_pool = ctx.enter_context(tc.tile_pool(name="xt", bufs=1))
        xt = xt_pool.tile([P, HCH, S], f32r, tag="xt")  # X^T, 64KB/partition
        v_pool = ctx.enter_context(tc.tile_pool(name="v", bufs=1))
        v = v_pool.tile([P, SCH, NHEADS, HD + 1], f32r, tag="v")  # V + ones col

        stage = ctx.enter_context(tc.tile_pool(name="stage", bufs=2))
        # PSUM: QK pair slots (2 x 2 banks), small slots for projections and
        # PE transposes (2 x 1 bank), AV accumulators (2 x 1 bank) = 8 banks.
        mmps = ctx.enter_context(tc.tile_pool(name="mmps", bufs=2, space="PSUM"))
        smps = ctx.enter_context(tc.tile_pool(name="smps", bufs=2, space="PSUM"))
        ops_ = ctx.enter_context(tc.tile_pool(name="ops", bufs=2, space="PSUM"))
        wt_pool = ctx.enter_context(tc.tile_pool(name="wt", bufs=2))
        qkt_pool = ctx.enter_context(tc.tile_pool(name="qkt", bufs=2))
        p_pool = ctx.enter_context(tc.tile_pool(name="pp", bufs=4))
        ot_pool = ctx.enter_context(tc.tile_pool(name="ot", bufs=2))
        obuf = ctx.enter_context(tc.tile_pool(name="obuf", bufs=4))
        rec_pool = ctx.enter_context(tc.tile_pool(name="rec", bufs=4))

        def transpose_one(dst_ap, src_ap):
            # dst[128, 128] (SBUF) = src[128, 128].T via PE + DVE copyback
            slot = smps.tile([P, F], f32, tag="sm")
            nc.tensor.transpose(slot[:, 0:128], src_ap, ident[:])
            nc.vector.tensor_copy(dst_ap, slot[:, 0:128])

        # ---- Wv^T (8 transposes per 128-row chunk of wv) ----
        with tc.tile_pool(name="wtv", bufs=1) as wtv_pool:
            wtv = wtv_pool.tile([P, HCH, F], f32r, tag="wtv")
            for rc in range(4):
                ws = stage.tile([P, H], f32, tag="stage")
                nc.sync.dma_start(ws[:], wv_d[rc * 128 : (rc + 1) * 128, :])
                for j in range(HCH):
                    transpose_one(
                        wtv[:, j, rc * 128 : (rc + 1) * 128],
                        ws[:, j * 128 : (j + 1) * 128],
                    )
            # ones column of V (tensor_scalar: in0*0 + 1; memset can't write f32r)
            nc.vector.tensor_scalar(
                v[:, :, :, HD : HD + 1],
                bvb[:, 0:128].rearrange("p (a b c) -> p a b c", a=SCH, b=NHEADS),
                0.0,
                1.0,
                MULT,
                ADD,
            )
            # ---- X^T interleaved with V = X @ Wv_s^T + bv ----
            for si in range(SCH):
                xs = stage.tile([P, H], f32, tag="stage")
                nc.sync.dma_start(xs[:], x_d[si * 128 : (si + 1) * 128, :])
                for j in range(HCH):
                    transpose_one(
                        xt[:, j, si * 128 : (si + 1) * 128],
                        xs[:, j * 128 : (j + 1) * 128],
                    )
                ps = mmps.tile([P, 1024], f32, tag="mm")
                for j in range(HCH):
                    nc.tensor.matmul(
                        ps[:, 0:F],
                        xt[:, j, si * 128 : (si + 1) * 128],
                        wtv[:, j, :],
                        start=(j == 0),
                        stop=(j == HCH - 1),
                    )
                nc.vector.tensor_tensor(
                    v[:, si, :, 0:HD],
                    ps[:, 0:F].rearrange("p (h d) -> p h d", h=NHEADS),
                    bvb[:].rearrange("p (h d) -> p h d", h=NHEADS),
                    ADD,
                )

        # ---- per head-pair: project Q^T/K^T, then attention ----
        # The projection work of pair p+1 is emitted in fine-grained units
        # interleaved into pair p's attention steps, placed between the
        # next QK prefetch and the exp-dependent AV matmuls so the PE has
        # independent work while ScalarE computes exp.
        def make_pair_proj(pr):
            wtq = wt_pool.tile([P, HCH, P], f32r, tag="wtq")
            wtk = wt_pool.tile([P, HCH, P], f32r, tag="wtk")
            qt = qkt_pool.tile([P, S], f32r, tag="qt")
            kt = qkt_pool.tile([P, S], f32r, tag="kt")
            units = []
            for wd, wt in ((wq_d, wtq), (wk_d, wtk)):
                ws = stage.tile([P, H], f32, tag="stage")

                def dma_u(ws=ws, wd=wd):
                    nc.sync.dma_start(ws[:], wd[pr * 128 : (pr + 1) * 128, :])

                units.append(dma_u)
                for j in range(HCH):

                    def tr_u(wt=wt, ws=ws, j=j):
                        transpose_one(wt[:, j, :], ws[:, j * 128 : (j + 1) * 128])

                    units.append(tr_u)
            for wt, dst, bias in ((wtq, qt, bqt), (wtk, kt, bkt)):
                for st in range(4):
                    ps = smps.tile([P, F], f32, tag="sm")
                    for j0 in range(0, HCH, 2):

                        def mm_u(wt=wt, ps=ps, st=st, j0=j0):
                            for j in (j0, j0 + 1):
                                nc.tensor.matmul(
                                    ps[:],
                                    wt[:, j, :],
                                    xt[:, j, st * 512 : (st + 1) * 512],
                                    start=(j == 0),
                                    stop=(j == HCH - 1),
                                )

                        units.append(mm_u)

                    def cb_u(dst=dst, ps=ps, st=st, bias=bias):
                        nc.vector.tensor_scalar_add(
                            dst[:, st * 512 : (st + 1) * 512],
                            ps[:],
                            bias[:, pr : pr + 1],
                        )

                    units.append(cb_u)
            return qt, kt, units

        pair_state = {0: make_pair_proj(0)}
        for u in pair_state[0][2]:
            u()

        for pr in range(NPAIR):
            qt, kt, _ = pair_state[pr]
            if pr + 1 < NPAIR:
                pair_state[pr + 1] = make_pair_proj(pr + 1)
                fill = pair_state[pr + 1][2]
            else:
                fill = []
            fi = [0]

            def emit_fill(n, fill=fill, fi=fi):
                while n > 0 and fi[0] < len(fill):
                    fill[fi[0]]()
                    fi[0] += 1
                    n -= 1

            h0, h1 = 2 * pr, 2 * pr + 1

            def emit_qk(qi, kc):
                q0 = qi * 512
                off = kc - 4 * qi
                lo = off * 128 if off > 0 else 0
                ps = mmps.tile([P, 1024], f32, tag="mm")
                nc.tensor.matmul(
                    ps[:, lo:512],
                    kt[0:64, kc * 128 : (kc + 1) * 128],
                    qt[0:64, q0 + lo : q0 + 512],
                    start=True,
                    stop=True,
                )
                nc.tensor.matmul(
                    ps[:, 512 + lo : 1024],
                    kt[64:128, kc * 128 : (kc + 1) * 128],
                    qt[64:128, q0 + lo : q0 + 512],
                    start=True,
                    stop=True,
                )
                return ps

            for qi in range(4):
                q0 = qi * 512
                nk = 4 * (qi + 1)
                oa = ops_.tile([P, F], f32, tag="o")
                ob = ops_.tile([P, F], f32, tag="o")
                ps = emit_qk(qi, 0)
                for kc in range(nk):
                    off = kc - 4 * qi
                    lo = off * 128 if off > 0 else 0
                    pt = p_pool.tile([P, 1024], f32r, tag="pt")
                    if lo == 0:
                        nc.scalar.activation(
                            pt[:], ps[:], EXP, bias=maskb[:, kc : kc + 1], scale=0.125
                        )
                    else:
                        nc.scalar.activation(
                            pt[:, lo:512],
                            ps[:, lo:512],
                            EXP,
                            bias=maskb[:, kc : kc + 1],
                            scale=0.125,
                        )
                        nc.scalar.activation(
                            pt[:, 512 + lo : 1024],
                            ps[:, 512 + lo : 1024],
                            EXP,
                            bias=maskb[:, kc : kc + 1],
                            scale=0.125,
                        )
                    if off >= 0:
                        nc.vector.tensor_mul(
                            pt[:, lo : lo + 128], pt[:, lo : lo + 128], cm[:]
                        )
                        nc.vector.tensor_mul(
                            pt[:, 512 + lo : 512 + lo + 128],
                            pt[:, 512 + lo : 512 + lo + 128],
                            cm[:],
                        )
                    if kc + 1 < nk:
                        ps = emit_qk(qi, kc + 1)
                    emit_fill(1)
                    nc.tensor.matmul(
                        oa[0 : HD + 1, lo:512],
                        v[:, kc, h0, :],
                        pt[:, lo:512],
                        start=(kc == 0),
                        stop=(kc == nk - 1),
                    )
                    nc.tensor.matmul(
                        ob[0 : HD + 1, lo:512],
                        v[:, kc, h1, :],
                        pt[:, 512 + lo : 1024],
                        start=(kc == 0),
                        stop=(kc == nk - 1),
                    )
                for o_ps, h in ((oa, h0), (ob, h1)):
                    ot = ot_pool.tile([HD + 1, F], f32, tag="ot")
                    nc.vector.tensor_copy(ot[:], o_ps[0 : HD + 1, :])
                    for t in range(4):
                        slot = smps.tile([P, F], f32, tag="sm")
                        nc.tensor.transpose(
                            slot[:, 0 : HD + 1],
                            ot[:, t * 128 : (t + 1) * 128],
                            ident[0 : HD + 1, 0 : HD + 1],
                        )
                        rec = rec_pool.tile([P, 1], f32, tag="rec")
                        nc.vector.reciprocal(rec[:], slot[:, HD : HD + 1])
                        otile = obuf.tile([P, HD], f32, tag="ob")
                        nc.vector.tensor_scalar_mul(
                            otile[:], slot[:, 0:HD], rec[:]
                        )
                        nc.sync.dma_start(
                            out_d[
                                q0 + t * 128 : q0 + (t + 1) * 128,
                                h * HD : (h + 1) * HD,
                            ],
                            otile[:],
                        )
            emit_fill(10**9)

    nc.compile()
    return nc


def _get_nc():
    if "nc" not in _CACHE:
        _CACHE["nc"] = _build_bass()
    return _CACHE["nc"]


def _host_consts():
    if "consts" not in _CACHE:
        qq = np.arange(P)[None, :]
        kk = np.arange(P)[:, None]
        _CACHE["consts"] = {
            "cm": (qq >= kk).astype(np.float32),
            "ident": np.eye(P, dtype=np.float32),
        }
    return _CACHE["consts"]


def make_in_maps(inputs):
    hs = np.asarray(inputs["hidden_states"], dtype=np.float32)
    am = np.asarray(inputs["attention_mask"], dtype=np.float32)
    Wq = np.asarray(inputs["Wq"], dtype=np.float32)
    bq = np.asarray(inputs["bq"], dtype=np.float32)
    Wk = np.asarray(inputs["Wk"], dtype=np.float32)
    bk = np.asarray(inputs["bk"], dtype=np.float32)
    Wv = np.asarray(inputs["Wv"], dtype=np.float32)
    bv = np.asarray(inputs["bv"], dtype=np.float32)

    consts = _host_consts()
    in_maps = []
    for c in range(NCORES):
        b, hg = c // 2, c % 2
        fsl = slice(hg * F, (hg + 1) * F)
        in_maps.append(
            {
                "x": np.ascontiguousarray(hs[b]),
                "wq": np.ascontiguousarray(Wq[fsl]),
                "wk": np.ascontiguousarray(Wk[fsl]),
                "wv": np.ascontiguousarray(Wv[fsl]),
                "bqt": np.ascontiguousarray(bq[fsl].reshape(NPAIR, P).T),
                "bkt": np.ascontiguousarray(bk[fsl].reshape(NPAIR, P).T),
                "bvb": np.broadcast_to(bv[fsl], (P, F)).copy(),
                "maskb": np.ascontiguousarray((am[b, 0, 0] / 8.0).reshape(SCH, P).T),
                "cm": consts["cm"],
                "ident": consts["ident"],
            }
        )
    return in_maps


def assemble_out(results):
    out = np.empty((B, S, H), dtype=np.float32)
    for c in range(NCORES):
        b, hg = c // 2, c % 2
        out[b, :, hg * F : (hg + 1) * F] = results[c]["out"]
    return out


def kernel(**inputs):
    from concourse.bass_utils import run_bass_kernel_spmd

    in_maps = make_in_maps(inputs)
    nc = _get_nc()
    res = run_bass_kernel_spmd(nc, in_maps, list(range(NCORES)))
    return assemble_out(res.results)


if __name__ == "__main__":
    rng = np.random.default_rng(0)
    ins = {
        "hidden_states": rng.standard_normal((B, S, H)).astype(np.float32),
        "attention_mask": np.zeros((B, 1, 1, S), np.float32),
        "Wq": (rng.standard_normal((H, H)) / 32.0).astype(np.float32),
        "bq": np.zeros(H, np.float32),
        "Wk": (rng.standard_normal((H, H)) / 32.0).astype(np.float32),
        "bk": np.zeros(H, np.float32),
        "Wv": (rng.standard_normal((H, H)) / 32.0).astype(np.float32),
        "bv": np.zeros(H, np.float32),
    }
    o = kernel(**ins)
    print("out", o.shape, o.dtype, float(np.abs(o).max()))


# revision 15
# speedup vs baseline: 1.1917x; 1.1183x over previous
"""Causal self-attention on 8 TRN2 NeuronCores.

Sharding: B=4 batches x 16 heads -> 64 (b,h) pairs; core c handles batch
b=c//2 and head-group hg=c%2 (8 heads = 512 of the 1024 features).
Q/K/V projection weights are row-sliced per head group (column-sharded in
the x @ W.T sense), so each core computes its own (b, 8-head) slice of the
S x S attention without any cross-core communication.

Kernel layout choices:
- Matmuls contract over SBUF partitions, so X^T (and W^T) are materialized
  on-chip via PE transpose-mode matmuls (fp32 has no DMA transpose). The
  X^T transposes are interleaved with the V-projection matmuls to keep the
  PE HAM clock-gate warm.
- All big matmuls run as float32r (full PE rate at free-dim >=256, ~tf32
  precision). Tiles feeding matmuls are allocated float32r so the
  producing op does the rounding (a bitcast is rejected by the verifier).
- Scores are computed transposed, S^T[k, q] = (K^T)^T Q^T per 128-key chunk,
  so that softmax(S)@V becomes out^T[d, q] = V^T P^T with a 512-wide moving
  operand. Heads are processed in pairs: head parity picks partitions 0-63
  vs 64-127 (independent PE row groups -> the two K=64 matmuls overlap).
- No row-max subtraction: scaled scores are ~N(0,1), exp is safe in fp32.
  exp runs on ScalarE straight from PSUM with the attention-mask bias and
  1/sqrt(64) scale fused in. Causal structure is exploited by narrowing
  the q-window of QK/exp/AV on diagonal tiles; only the single partial
  128-col window needs a 0/1 mask multiply after exp.
- A ones-column appended to V makes the AV matmul also produce the softmax
  denominator (row 64 of the [65, 512] PSUM accumulator).
- Output heads are PE-transposed back to [seq, d] and normalized by the
  reciprocal of the denominator column on the way out.
"""

import sys

if "/opt/trn_rl_repo" not in sys.path:
    sys.path.insert(0, "/opt/trn_rl_repo")

import numpy as np
import ml_dtypes

_bf16 = np.dtype(ml_dtypes.bfloat16)

B, S, H, NH = 4, 2048, 1024, 16
HD = 64
NCORES = 8
F = 512  # features per core (8 heads)
NHEADS = 8  # heads per core
NPAIR = 4  # head pairs per core
HCH = H // 128  # 8 hidden chunks
SCH = S // 128  # 16 sequence chunks
P = 128

_CACHE = {}


def _build_bass():
    import concourse.tile as tile
    from concourse import bacc, mybir
    from contextlib import ExitStack

    f32 = mybir.dt.float32
    f32r = mybir.dt.float32r
    EXP = mybir.ActivationFunctionType.Exp
    ADD = mybir.AluOpType.add
    MULT = mybir.AluOpType.mult

    nc = bacc.Bacc("TRN2", target_bir_lowering=False, debug=False, num_devices=NCORES)

    bf16 = mybir.dt.bfloat16
    x_d = nc.dram_tensor("xb", [S, H], bf16, kind="ExternalInput").ap()
    wq_d = nc.dram_tensor("wqb", [F, H], bf16, kind="ExternalInput").ap()
    wk_d = nc.dram_tensor("wkb", [F, H], bf16, kind="ExternalInput").ap()
    wv_d = nc.dram_tensor("wvb", [F, H], bf16, kind="ExternalInput").ap()
    bqt_d = nc.dram_tensor("bqt", [P, NPAIR], f32, kind="ExternalInput").ap()
    bkt_d = nc.dram_tensor("bkt", [P, NPAIR], f32, kind="ExternalInput").ap()
    bvb_d = nc.dram_tensor("bvb", [P, F], f32, kind="ExternalInput").ap()
    maskb_d = nc.dram_tensor("maskb", [P, SCH], f32, kind="ExternalInput").ap()
    cm_d = nc.dram_tensor("cm", [P, P], f32, kind="ExternalInput").ap()
    id_d = nc.dram_tensor("ident", [P, P], f32, kind="ExternalInput").ap()
    out_d = nc.dram_tensor("out", [S, F], f32, kind="ExternalOutput").ap()

    with tile.TileContext(nc) as tc, ExitStack() as ctx:
        const = ctx.enter_context(tc.tile_pool(name="const", bufs=1))
        ident = const.tile([P, P], f32, tag="ident")
        nc.sync.dma_start(ident[:], id_d[:])
        cm = const.tile([P, P], f32, tag="cm")
        nc.sync.dma_start(cm[:], cm_d[:])
        maskb = const.tile([P, SCH], f32, tag="maskb")
        nc.sync.dma_start(maskb[:], maskb_d[:])
        bqt = const.tile([P, NPAIR], f32, tag="bqt")
        nc.sync.dma_start(bqt[:], bqt_d[:])
        bkt = const.tile([P, NPAIR], f32, tag="bkt")
        nc.sync.dma_start(bkt[:], bkt_d[:])
        bvb = const.tile([P, F], f32, tag="bvb")
        nc.sync.dma_start(bvb[:], bvb_d[:])

        xt_pool = ctx.enter_context(tc.tile_pool(name="xt", bufs=1))
        xt = xt_pool.tile([P, HCH, S], bf16, tag="xt")  # X^T via DMA transpose
        v_pool = ctx.enter_context(tc.tile_pool(name="v", bufs=1))
        v = v_pool.tile([P, SCH, NHEADS, HD + 1], f32r, tag="v")  # V + ones col

        # PSUM: QK pair slots (2 x 2 banks), small slots for projections and
        # PE transposes (2 x 1 bank), AV accumulators (2 x 1 bank) = 8 banks.
        mmps = ctx.enter_context(tc.tile_pool(name="mmps", bufs=2, space="PSUM"))
        smps = ctx.enter_context(tc.tile_pool(name="smps", bufs=2, space="PSUM"))
        ops_ = ctx.enter_context(tc.tile_pool(name="ops", bufs=2, space="PSUM"))
        wt_pool = ctx.enter_context(tc.tile_pool(name="wt", bufs=2))
        qkt_pool = ctx.enter_context(tc.tile_pool(name="qkt", bufs=2))
        p_pool = ctx.enter_context(tc.tile_pool(name="pp", bufs=6))
        ot_pool = ctx.enter_context(tc.tile_pool(name="ot", bufs=2))
        obuf = ctx.enter_context(tc.tile_pool(name="obuf", bufs=4))
        rec_pool = ctx.enter_context(tc.tile_pool(name="rec", bufs=4))

        # ---- Wv^T and X^T via bf16 DMA transpose (XBAR fast path) ----
        with tc.tile_pool(name="wtv", bufs=1) as wtv_pool:
            wtv = wtv_pool.tile([P, HCH, F], bf16, tag="wtv")
            for j in range(HCH):
                nc.sync.dma_start_transpose(
                    wtv[:, j, :], wv_d[:, j * 128 : (j + 1) * 128]
                )
                nc.sync.dma_start_transpose(
                    xt[:, j, :], x_d[:, j * 128 : (j + 1) * 128]
                )
            # ones column of V (tensor_scalar: in0*0 + 1; memset can't write f32r)
            nc.vector.tensor_scalar(
                v[:, :, :, HD : HD + 1],
                bvb[:, 0:128].rearrange("p (a b c) -> p a b c", a=SCH, b=NHEADS),
                0.0,
                1.0,
                MULT,
                ADD,
            )
            # ---- V = X @ Wv_s^T + bv ----
            for si in range(SCH):
                ps = mmps.tile([P, 1024], f32, tag="mm")
                for j in range(HCH):
                    nc.tensor.matmul(
                        ps[:, 0:F],
                        xt[:, j, si * 128 : (si + 1) * 128],
                        wtv[:, j, :],
                        start=(j == 0),
                        stop=(j == HCH - 1),
                    )
                nc.vector.tensor_tensor(
                    v[:, si, :, 0:HD],
                    ps[:, 0:F].rearrange("p (h d) -> p h d", h=NHEADS),
                    bvb[:].rearrange("p (h d) -> p h d", h=NHEADS),
                    ADD,
                )

        # ---- per head-pair: project Q^T/K^T, then attention ----
        # The projection work of pair p+1 is emitted in fine-grained units
        # interleaved into pair p's attention steps, placed between the
        # next QK prefetch and the exp-dependent AV matmuls so the PE has
        # independent work while ScalarE computes exp.
        def make_pair_proj(pr):
            wtq = wt_pool.tile([P, HCH, P], bf16, tag="wtq")
            wtk = wt_pool.tile([P, HCH, P], bf16, tag="wtk")
            qt = qkt_pool.tile([P, S], f32r, tag="qt")
            kt = qkt_pool.tile([P, S], f32r, tag="kt")
            units = []
            for wd, wt in ((wq_d, wtq), (wk_d, wtk)):

                def dma_u(wt=wt, wd=wd):
                    for j in range(HCH):
                        nc.sync.dma_start_transpose(
                            wt[:, j, :],
                            wd[pr * 128 : (pr + 1) * 128, j * 128 : (j + 1) * 128],
                        )

                units.append(dma_u)
            for wt, dst, bias in ((wtq, qt, bqt), (wtk, kt, bkt)):
                for st in range(4):
                    ps = smps.tile([P, F], f32, tag="sm")
                    for j0 in range(0, HCH, 2):

                        def mm_u(wt=wt, ps=ps, st=st, j0=j0):
                            for j in (j0, j0 + 1):
                                nc.tensor.matmul(
                                    ps[:],
                                    wt[:, j, :],
                                    xt[:, j, st * 512 : (st + 1) * 512],
                                    start=(j == 0),
                                    stop=(j == HCH - 1),
                                )

                        units.append(mm_u)

                    def cb_u(dst=dst, ps=ps, st=st, bias=bias):
                        nc.vector.tensor_scalar_add(
                            dst[:, st * 512 : (st + 1) * 512],
                            ps[:],
                            bias[:, pr : pr + 1],
                        )

                    units.append(cb_u)
            return qt, kt, units

        pair_state = {0: make_pair_proj(0)}
        for u in pair_state[0][2]:
            u()

        for pr in range(NPAIR):
            qt, kt, _ = pair_state[pr]
            if pr + 1 < NPAIR:
                pair_state[pr + 1] = make_pair_proj(pr + 1)
                fill = pair_state[pr + 1][2]
            else:
                fill = []
            fi = [0]

            def emit_fill(n, fill=fill, fi=fi):
                while n > 0 and fi[0] < len(fill):
                    fill[fi[0]]()
                    fi[0] += 1
                    n -= 1

            h0, h1 = 2 * pr, 2 * pr + 1

            def emit_qk(qi, kc):
                q0 = qi * 512
                off = kc - 4 * qi
                lo = off * 128 if off > 0 else 0
                ps = mmps.tile([P, 1024], f32, tag="mm")
                nc.tensor.matmul(
                    ps[:, lo:512],
                    kt[0:64, kc * 128 : (kc + 1) * 128],
                    qt[0:64, q0 + lo : q0 + 512],
                    start=True,
                    stop=True,
                )
                nc.tensor.matmul(
                    ps[:, 512 + lo : 1024],
                    kt[64:128, kc * 128 : (kc + 1) * 128],
                    qt[64:128, q0 + lo : q0 + 512],
                    start=True,
                    stop=True,
                )
                return ps

            for qi in range(4):
                q0 = qi * 512
                nk = 4 * (qi + 1)
                oa = ops_.tile([P, F], f32, tag="o")
                ob = ops_.tile([P, F], f32, tag="o")
                ps = emit_qk(qi, 0)
                for kc in range(nk):
                    off = kc - 4 * qi
                    lo = off * 128 if off > 0 else 0
                    pt = p_pool.tile([P, 1024], f32r, tag="pt")
                    if lo == 0:
                        nc.scalar.activation(
                            pt[:], ps[:], EXP, bias=maskb[:, kc : kc + 1], scale=0.125
                        )
                    else:
                        nc.scalar.activation(
                            pt[:, lo:512],
                            ps[:, lo:512],
                            EXP,
                            bias=maskb[:, kc : kc + 1],
                            scale=0.125,
                        )
                        nc.scalar.activation(
                            pt[:, 512 + lo : 1024],
                            ps[:, 512 + lo : 1024],
                            EXP,
                            bias=maskb[:, kc : kc + 1],
                            scale=0.125,
                        )
                    if off >= 0:
                        nc.vector.tensor_mul(
                            pt[:, lo : lo + 128], pt[:, lo : lo + 128], cm[:]
                        )
                        nc.vector.tensor_mul(
                            pt[:, 512 + lo : 512 + lo + 128],
                            pt[:, 512 + lo : 512 + lo + 128],
                            cm[:],
                        )
                    if kc + 1 < nk:
                        ps = emit_qk(qi, kc + 1)
                    emit_fill(1)
                    nc.tensor.matmul(
                        oa[0 : HD + 1, lo:512],
                        v[:, kc, h0, :],
                        pt[:, lo:512],
                        start=(kc == 0),
                        stop=(kc == nk - 1),
                    )
                    nc.tensor.matmul(
                        ob[0 : HD + 1, lo:512],
                        v[:, kc, h1, :],
                        pt[:, 512 + lo : 1024],
                        start=(kc == 0),
                        stop=(kc == nk - 1),
                    )
                for o_ps, h in ((oa, h0), (ob, h1)):
                    ot = ot_pool.tile([HD + 1, F], f32, tag="ot")
                    nc.vector.tensor_copy(ot[:], o_ps[0 : HD + 1, :])
                    for t in range(4):
                        slot = smps.tile([P, F], f32, tag="sm")
                        nc.tensor.transpose(
                            slot[:, 0 : HD + 1],
                            ot[:, t * 128 : (t + 1) * 128],
                            ident[0 : HD + 1, 0 : HD + 1],
                        )
                        rec = rec_pool.tile([P, 1], f32, tag="rec")
                        nc.vector.reciprocal(rec[:], slot[:, HD : HD + 1])
                        otile = obuf.tile([P, HD], f32, tag="ob")
                        nc.vector.tensor_scalar_mul(
                            otile[:], slot[:, 0:HD], rec[:]
                        )
                        nc.sync.dma_start(
                            out_d[
                                q0 + t * 128 : q0 + (t + 1) * 128,
                                h * HD : (h + 1) * HD,
                            ],
                            otile[:],
                        )
            emit_fill(10**9)

    nc.compile()
    return nc


def _get_nc():
    if "nc" not in _CACHE:
        _CACHE["nc"] = _build_bass()
    return _CACHE["nc"]


def _host_consts():
    if "consts" not in _CACHE:
        qq = np.arange(P)[None, :]
        kk = np.arange(P)[:, None]
        _CACHE["consts"] = {
            "cm": (qq >= kk).astype(np.float32),
            "ident": np.eye(P, dtype=np.float32),
        }
    return _CACHE["consts"]


def make_in_maps(inputs):
    hs = np.asarray(inputs["hidden_states"], dtype=np.float32)
    am = np.asarray(inputs["attention_mask"], dtype=np.float32)
    Wq = np.asarray(inputs["Wq"], dtype=np.float32)
    bq = np.asarray(inputs["bq"], dtype=np.float32)
    Wk = np.asarray(inputs["Wk"], dtype=np.float32)
    bk = np.asarray(inputs["bk"], dtype=np.float32)
    Wv = np.asarray(inputs["Wv"], dtype=np.float32)
    bv = np.asarray(inputs["bv"], dtype=np.float32)

    consts = _host_consts()
    in_maps = []
    for c in range(NCORES):
        b, hg = c // 2, c % 2
        fsl = slice(hg * F, (hg + 1) * F)
        in_maps.append(
            {
                "xb": np.ascontiguousarray(hs[b]).astype(_bf16),
                "wqb": np.ascontiguousarray(Wq[fsl]).astype(_bf16),
                "wkb": np.ascontiguousarray(Wk[fsl]).astype(_bf16),
                "wvb": np.ascontiguousarray(Wv[fsl]).astype(_bf16),
                "bqt": np.ascontiguousarray(bq[fsl].reshape(NPAIR, P).T),
                "bkt": np.ascontiguousarray(bk[fsl].reshape(NPAIR, P).T),
                "bvb": np.broadcast_to(bv[fsl], (P, F)).copy(),
                "maskb": np.ascontiguousarray((am[b, 0, 0] / 8.0).reshape(SCH, P).T),
                "cm": consts["cm"],
                "ident": consts["ident"],
            }
        )
    return in_maps


def assemble_out(results):
    out = np.empty((B, S, H), dtype=np.float32)
    for c in range(NCORES):
        b, hg = c // 2, c % 2
        out[b, :, hg * F : (hg + 1) * F] = results[c]["out"]
    return out


def kernel(**inputs):
    from concourse.bass_utils import run_bass_kernel_spmd

    in_maps = make_in_maps(inputs)
    nc = _get_nc()
    res = run_bass_kernel_spmd(nc, in_maps, list(range(NCORES)))
    return assemble_out(res.results)


if __name__ == "__main__":
    rng = np.random.default_rng(0)
    ins = {
        "hidden_states": rng.standard_normal((B, S, H)).astype(np.float32),
        "attention_mask": np.zeros((B, 1, 1, S), np.float32),
        "Wq": (rng.standard_normal((H, H)) / 32.0).astype(np.float32),
        "bq": np.zeros(H, np.float32),
        "Wk": (rng.standard_normal((H, H)) / 32.0).astype(np.float32),
        "bk": np.zeros(H, np.float32),
        "Wv": (rng.standard_normal((H, H)) / 32.0).astype(np.float32),
        "bv": np.zeros(H, np.float32),
    }
    o = kernel(**ins)
    print("out", o.shape, o.dtype, float(np.abs(o).max()))


# revision 18
# speedup vs baseline: 1.2592x; 1.0566x over previous
"""Causal self-attention on 8 TRN2 NeuronCores.

Sharding: B=4 batches x 16 heads -> 64 (b,h) pairs; core c handles batch
b=c//2 and head-group hg=c%2 (8 heads = 512 of the 1024 features).
Q/K/V projection weights are row-sliced per head group (column-sharded in
the x @ W.T sense), so each core computes its own (b, 8-head) slice of the
S x S attention without any cross-core communication.

Kernel layout choices:
- Matmuls contract over SBUF partitions, so X^T (and W^T) are materialized
  on-chip via PE transpose-mode matmuls (fp32 has no DMA transpose). The
  X^T transposes are interleaved with the V-projection matmuls to keep the
  PE HAM clock-gate warm.
- All big matmuls run as float32r (full PE rate at free-dim >=256, ~tf32
  precision). Tiles feeding matmuls are allocated float32r so the
  producing op does the rounding (a bitcast is rejected by the verifier).
- Scores are computed transposed, S^T[k, q] = (K^T)^T Q^T per 128-key chunk,
  so that softmax(S)@V becomes out^T[d, q] = V^T P^T with a 512-wide moving
  operand. Heads are processed in pairs: head parity picks partitions 0-63
  vs 64-127 (independent PE row groups -> the two K=64 matmuls overlap).
- No row-max subtraction: scaled scores are ~N(0,1), exp is safe in fp32.
  exp runs on ScalarE straight from PSUM with the attention-mask bias and
  1/sqrt(64) scale fused in. Causal structure is exploited by narrowing
  the q-window of QK/exp/AV on diagonal tiles; only the single partial
  128-col window needs a 0/1 mask multiply after exp.
- A ones-column appended to V makes the AV matmul also produce the softmax
  denominator (row 64 of the [65, 512] PSUM accumulator).
- Output heads are PE-transposed back to [seq, d] and normalized by the
  reciprocal of the denominator column on the way out.
"""

import sys

if "/opt/trn_rl_repo" not in sys.path:
    sys.path.insert(0, "/opt/trn_rl_repo")

import numpy as np
import ml_dtypes

_bf16 = np.dtype(ml_dtypes.bfloat16)

B, S, H, NH = 4, 2048, 1024, 16
HD = 64
NCORES = 8
F = 512  # features per core (8 heads)
NHEADS = 8  # heads per core
NPAIR = 4  # head pairs per core
HCH = H // 128  # 8 hidden chunks
SCH = S // 128  # 16 sequence chunks
P = 128

_CACHE = {}


def _build_bass():
    import concourse.tile as tile
    from concourse import bacc, mybir
    from contextlib import ExitStack

    f32 = mybir.dt.float32
    f32r = mybir.dt.float32r
    EXP = mybir.ActivationFunctionType.Exp
    ADD = mybir.AluOpType.add
    MULT = mybir.AluOpType.mult

    nc = bacc.Bacc("TRN2", target_bir_lowering=False, debug=False, num_devices=NCORES)

    bf16 = mybir.dt.bfloat16
    x_d = nc.dram_tensor("xb", [S, H], bf16, kind="ExternalInput").ap()
    wq_d = nc.dram_tensor("wqb", [F, H], bf16, kind="ExternalInput").ap()
    wk_d = nc.dram_tensor("wkb", [F, H], bf16, kind="ExternalInput").ap()
    wv_d = nc.dram_tensor("wvb", [F, H], bf16, kind="ExternalInput").ap()
    bqt_d = nc.dram_tensor("bqt", [P, NPAIR], f32, kind="ExternalInput").ap()
    bkt_d = nc.dram_tensor("bkt", [P, NPAIR], f32, kind="ExternalInput").ap()
    bvb_d = nc.dram_tensor("bvb", [P, F], f32, kind="ExternalInput").ap()
    maskb_d = nc.dram_tensor("maskb", [P, SCH], f32, kind="ExternalInput").ap()
    cm_d = nc.dram_tensor("cm", [P, P], bf16, kind="ExternalInput").ap()
    id_d = nc.dram_tensor("ident", [P, P], f32, kind="ExternalInput").ap()
    out_d = nc.dram_tensor("out", [S, F], f32, kind="ExternalOutput").ap()

    with tile.TileContext(nc) as tc, ExitStack() as ctx:
        const = ctx.enter_context(tc.tile_pool(name="const", bufs=1))
        ident = const.tile([P, P], f32, tag="ident")
        nc.sync.dma_start(ident[:], id_d[:])
        cm = const.tile([P, P], bf16, tag="cm")
        nc.sync.dma_start(cm[:], cm_d[:])
        maskb = const.tile([P, SCH], f32, tag="maskb")
        nc.sync.dma_start(maskb[:], maskb_d[:])
        bqt = const.tile([P, NPAIR], f32, tag="bqt")
        nc.sync.dma_start(bqt[:], bqt_d[:])
        bkt = const.tile([P, NPAIR], f32, tag="bkt")
        nc.sync.dma_start(bkt[:], bkt_d[:])
        bvb = const.tile([P, F], f32, tag="bvb")
        nc.sync.dma_start(bvb[:], bvb_d[:])

        xt_pool = ctx.enter_context(tc.tile_pool(name="xt", bufs=1))
        xt = xt_pool.tile([P, HCH, S], bf16, tag="xt")  # X^T via DMA transpose
        v_pool = ctx.enter_context(tc.tile_pool(name="v", bufs=1))
        v = v_pool.tile([P, SCH, NHEADS, HD + 1], bf16, tag="v")  # V + ones col

        # PSUM: QK pair slots (2 x 2 banks), small slots for projections and
        # PE transposes (2 x 1 bank), AV accumulators (2 x 1 bank) = 8 banks.
        mmps = ctx.enter_context(tc.tile_pool(name="mmps", bufs=2, space="PSUM"))
        smps = ctx.enter_context(tc.tile_pool(name="smps", bufs=2, space="PSUM"))
        ops_ = ctx.enter_context(tc.tile_pool(name="ops", bufs=2, space="PSUM"))
        wt_pool = ctx.enter_context(tc.tile_pool(name="wt", bufs=2))
        qkt_pool = ctx.enter_context(tc.tile_pool(name="qkt", bufs=2))
        p_pool = ctx.enter_context(tc.tile_pool(name="pp", bufs=6))
        ot_pool = ctx.enter_context(tc.tile_pool(name="ot", bufs=2))
        obuf = ctx.enter_context(tc.tile_pool(name="obuf", bufs=4))
        rec_pool = ctx.enter_context(tc.tile_pool(name="rec", bufs=4))

        # ---- per head-pair: project Q^T/K^T, then attention ----
        # The projection work of pair p+1 is emitted in fine-grained units
        # interleaved into pair p's attention steps, placed between the
        # next QK prefetch and the exp-dependent AV matmuls so the PE has
        # independent work while ScalarE computes exp.
        def make_pair_proj(pr):
            wtq = wt_pool.tile([P, HCH, P], bf16, tag="wtq")
            wtk = wt_pool.tile([P, HCH, P], bf16, tag="wtk")
            qt = qkt_pool.tile([P, S], bf16, tag="qt")
            kt = qkt_pool.tile([P, S], bf16, tag="kt")
            units = []
            for wd, wt in ((wq_d, wtq), (wk_d, wtk)):

                def dma_u(wt=wt, wd=wd):
                    for j in range(HCH):
                        nc.sync.dma_start_transpose(
                            wt[:, j, :],
                            wd[pr * 128 : (pr + 1) * 128, j * 128 : (j + 1) * 128],
                        )

                units.append(dma_u)
            for wt, dst, bias in ((wtq, qt, bqt), (wtk, kt, bkt)):
                for st in range(4):
                    ps = smps.tile([P, F], f32, tag="sm")
                    for j0 in range(0, HCH, 2):

                        def mm_u(wt=wt, ps=ps, st=st, j0=j0):
                            for j in (j0, j0 + 1):
                                nc.tensor.matmul(
                                    ps[:],
                                    wt[:, j, :],
                                    xt[:, j, st * 512 : (st + 1) * 512],
                                    start=(j == 0),
                                    stop=(j == HCH - 1),
                                )

                        units.append(mm_u)

                    def cb_u(dst=dst, ps=ps, st=st, bias=bias):
                        nc.vector.tensor_scalar_add(
                            dst[:, st * 512 : (st + 1) * 512],
                            ps[:],
                            bias[:, pr : pr + 1],
                        )

                    units.append(cb_u)
            return qt, kt, units

        # ---- A0: Wv^T/X^T DMA transposes interleaved with V-projection and
        # pair-0's Q^T/K^T projection, per 512-wide s-block, so the PE has
        # matmul work as soon as the first quarter of X^T lands. ----
        wtv = wt_pool.tile([P, HCH, F], bf16, tag="wtv")
        for j in range(HCH):
            nc.sync.dma_start_transpose(wtv[:, j, :], wv_d[:, j * 128 : (j + 1) * 128])
        nc.vector.tensor_scalar(
            v[:, :, :, HD : HD + 1],
            bvb[:, 0:128].rearrange("p (a b c) -> p a b c", a=SCH, b=NHEADS),
            0.0,
            1.0,
            MULT,
            ADD,
        )
        pair_state = {0: make_pair_proj(0)}
        p0u = pair_state[0][2]
        assert len(p0u) == 42
        p0u[0]()
        p0u[1]()
        for sb in range(4):
            for j in range(HCH):
                nc.sync.dma_start_transpose(
                    xt[:, j, sb * 512 : (sb + 1) * 512],
                    x_d[sb * 512 : (sb + 1) * 512, j * 128 : (j + 1) * 128],
                )
            for si in range(4 * sb, 4 * sb + 4):
                ps = mmps.tile([P, 1024], f32, tag="mm")
                for j in range(HCH):
                    nc.tensor.matmul(
                        ps[:, 0:F],
                        xt[:, j, si * 128 : (si + 1) * 128],
                        wtv[:, j, :],
                        start=(j == 0),
                        stop=(j == HCH - 1),
                    )
                nc.vector.tensor_tensor(
                    v[:, si, :, 0:HD],
                    ps[:, 0:F].rearrange("p (h d) -> p h d", h=NHEADS),
                    bvb[:].rearrange("p (h d) -> p h d", h=NHEADS),
                    ADD,
                )
            for u in p0u[2 + 5 * sb : 7 + 5 * sb]:
                u()
            for u in p0u[22 + 5 * sb : 27 + 5 * sb]:
                u()

        for pr in range(NPAIR):
            qt, kt, _ = pair_state[pr]
            if pr + 1 < NPAIR:
                pair_state[pr + 1] = make_pair_proj(pr + 1)
                fill = pair_state[pr + 1][2]
            else:
                fill = []
            fi = [0]

            def emit_fill(n, fill=fill, fi=fi):
                while n > 0 and fi[0] < len(fill):
                    fill[fi[0]]()
                    fi[0] += 1
                    n -= 1

            h0, h1 = 2 * pr, 2 * pr + 1

            def emit_qk(qi, kc):
                q0 = qi * 512
                off = kc - 4 * qi
                lo = off * 128 if off > 0 else 0
                ps = mmps.tile([P, 1024], f32, tag="mm")
                nc.tensor.matmul(
                    ps[:, lo:512],
                    kt[0:64, kc * 128 : (kc + 1) * 128],
                    qt[0:64, q0 + lo : q0 + 512],
                    start=True,
                    stop=True,
                )
                nc.tensor.matmul(
                    ps[:, 512 + lo : 1024],
                    kt[64:128, kc * 128 : (kc + 1) * 128],
                    qt[64:128, q0 + lo : q0 + 512],
                    start=True,
                    stop=True,
                )
                return ps

            for qi in range(4):
                q0 = qi * 512
                nk = 4 * (qi + 1)
                oa = ops_.tile([P, F], f32, tag="o")
                ob = ops_.tile([P, F], f32, tag="o")
                ps = emit_qk(qi, 0)
                for kc in range(nk):
                    off = kc - 4 * qi
                    lo = off * 128 if off > 0 else 0
                    pt = p_pool.tile([P, 1024], bf16, tag="pt")
                    if lo == 0:
                        nc.scalar.activation(
                            pt[:], ps[:], EXP, bias=maskb[:, kc : kc + 1], scale=0.125
                        )
                    else:
                        nc.scalar.activation(
                            pt[:, lo:512],
                            ps[:, lo:512],
                            EXP,
                            bias=maskb[:, kc : kc + 1],
                            scale=0.125,
                        )
                        nc.scalar.activation(
                            pt[:, 512 + lo : 1024],
                            ps[:, 512 + lo : 1024],
                            EXP,
                            bias=maskb[:, kc : kc + 1],
                            scale=0.125,
                        )
                    if off >= 0:
                        nc.vector.tensor_mul(
                            pt[:, lo : lo + 128], pt[:, lo : lo + 128], cm[:]
                        )
                        nc.vector.tensor_mul(
                            pt[:, 512 + lo : 512 + lo + 128],
                            pt[:, 512 + lo : 512 + lo + 128],
                            cm[:],
                        )
                    if kc + 1 < nk:
                        ps = emit_qk(qi, kc + 1)
                    emit_fill(1)
                    nc.tensor.matmul(
                        oa[0 : HD + 1, lo:512],
                        v[:, kc, h0, :],
                        pt[:, lo:512],
                        start=(kc == 0),
                        stop=(kc == nk - 1),
                    )
                    nc.tensor.matmul(
                        ob[0 : HD + 1, lo:512],
                        v[:, kc, h1, :],
                        pt[:, 512 + lo : 1024],
                        start=(kc == 0),
                        stop=(kc == nk - 1),
                    )
                for o_ps, h in ((oa, h0), (ob, h1)):
                    ot = ot_pool.tile([HD + 1, F], f32, tag="ot")
                    nc.vector.tensor_copy(ot[:], o_ps[0 : HD + 1, :])
                    for t in range(4):
                        slot = smps.tile([P, F], f32, tag="sm")
                        nc.tensor.transpose(
                            slot[:, 0 : HD + 1],
                            ot[:, t * 128 : (t + 1) * 128],
                            ident[0 : HD + 1, 0 : HD + 1],
                        )
                        rec = rec_pool.tile([P, 1], f32, tag="rec")
                        nc.vector.reciprocal(rec[:], slot[:, HD : HD + 1])
                        otile = obuf.tile([P, HD], f32, tag="ob")
                        nc.vector.tensor_scalar_mul(
                            otile[:], slot[:, 0:HD], rec[:]
                        )
                        nc.sync.dma_start(
                            out_d[
                                q0 + t * 128 : q0 + (t + 1) * 128,
                                h * HD : (h + 1) * HD,
                            ],
                            otile[:],
                        )
            emit_fill(10**9)

    nc.compile()
    return nc


def _get_nc():
    if "nc" not in _CACHE:
        _CACHE["nc"] = _build_bass()
    return _CACHE["nc"]


def _host_consts():
    if "consts" not in _CACHE:
        qq = np.arange(P)[None, :]
        kk = np.arange(P)[:, None]
        _CACHE["consts"] = {
            "cm": (qq >= kk).astype(_bf16),
            "ident": np.eye(P, dtype=np.float32),
        }
    return _CACHE["consts"]


def make_in_maps(inputs):
    hs = np.asarray(inputs["hidden_states"], dtype=np.float32)
    am = np.asarray(inputs["attention_mask"], dtype=np.float32)
    Wq = np.asarray(inputs["Wq"], dtype=np.float32)
    bq = np.asarray(inputs["bq"], dtype=np.float32)
    Wk = np.asarray(inputs["Wk"], dtype=np.float32)
    bk = np.asarray(inputs["bk"], dtype=np.float32)
    Wv = np.asarray(inputs["Wv"], dtype=np.float32)
    bv = np.asarray(inputs["bv"], dtype=np.float32)

    consts = _host_consts()
    in_maps = []
    for c in range(NCORES):
        b, hg = c // 2, c % 2
        fsl = slice(hg * F, (hg + 1) * F)
        in_maps.append(
            {
                "xb": np.ascontiguousarray(hs[b]).astype(_bf16),
                "wqb": np.ascontiguousarray(Wq[fsl]).astype(_bf16),
                "wkb": np.ascontiguousarray(Wk[fsl]).astype(_bf16),
                "wvb": np.ascontiguousarray(Wv[fsl]).astype(_bf16),
                "bqt": np.ascontiguousarray(bq[fsl].reshape(NPAIR, P).T),
                "bkt": np.ascontiguousarray(bk[fsl].reshape(NPAIR, P).T),
                "bvb": np.broadcast_to(bv[fsl], (P, F)).copy(),
                "maskb": np.ascontiguousarray((am[b, 0, 0] / 8.0).reshape(SCH, P).T),
                "cm": consts["cm"],
                "ident": consts["ident"],
            }
        )
    return in_maps


def assemble_out(results):
    out = np.empty((B, S, H), dtype=np.float32)
    for c in range(NCORES):
        b, hg = c // 2, c % 2
        out[b, :, hg * F : (hg + 1) * F] = results[c]["out"]
    return out


def kernel(**inputs):
    from concourse.bass_utils import run_bass_kernel_spmd

    in_maps = make_in_maps(inputs)
    nc = _get_nc()
    res = run_bass_kernel_spmd(nc, in_maps, list(range(NCORES)))
    return assemble_out(res.results)


if __name__ == "__main__":
    rng = np.random.default_rng(0)
    ins = {
        "hidden_states": rng.standard_normal((B, S, H)).astype(np.float32),
        "attention_mask": np.zeros((B, 1, 1, S), np.float32),
        "Wq": (rng.standard_normal((H, H)) / 32.0).astype(np.float32),
        "bq": np.zeros(H, np.float32),
        "Wk": (rng.standard_normal((H, H)) / 32.0).astype(np.float32),
        "bk": np.zeros(H, np.float32),
        "Wv": (rng.standard_normal((H, H)) / 32.0).astype(np.float32),
        "bv": np.zeros(H, np.float32),
    }
    o = kernel(**ins)
    print("out", o.shape, o.dtype, float(np.abs(o).max()))


# revision 20
# speedup vs baseline: 1.4266x; 1.1329x over previous
"""Causal self-attention on 8 TRN2 NeuronCores.

Sharding: B=4 batches x 16 heads -> 64 (b,h) pairs; core c handles batch
b=c//2 and head-group hg=c%2 (8 heads = 512 of the 1024 features).
Q/K/V projection weights are row-sliced per head group (column-sharded in
the x @ W.T sense), so each core computes its own (b, 8-head) slice of the
S x S attention without any cross-core communication.

Kernel layout choices:
- Matmuls contract over SBUF partitions, so X^T (and W^T) are materialized
  on-chip via PE transpose-mode matmuls (fp32 has no DMA transpose). The
  X^T transposes are interleaved with the V-projection matmuls to keep the
  PE HAM clock-gate warm.
- All big matmuls run as float32r (full PE rate at free-dim >=256, ~tf32
  precision). Tiles feeding matmuls are allocated float32r so the
  producing op does the rounding (a bitcast is rejected by the verifier).
- Scores are computed transposed, S^T[k, q] = (K^T)^T Q^T per 128-key chunk,
  so that softmax(S)@V becomes out^T[d, q] = V^T P^T with a 512-wide moving
  operand. Heads are processed in pairs: head parity picks partitions 0-63
  vs 64-127 (independent PE row groups -> the two K=64 matmuls overlap).
- No row-max subtraction: scaled scores are ~N(0,1), exp is safe in fp32.
  exp runs on ScalarE straight from PSUM with the attention-mask bias and
  1/sqrt(64) scale fused in. Causal structure is exploited by narrowing
  the q-window of QK/exp/AV on diagonal tiles; only the single partial
  128-col window needs a 0/1 mask multiply after exp.
- A ones-column appended to V makes the AV matmul also produce the softmax
  denominator (row 64 of the [65, 512] PSUM accumulator).
- Output heads are PE-transposed back to [seq, d] and normalized by the
  reciprocal of the denominator column on the way out.
"""

import sys

if "/opt/trn_rl_repo" not in sys.path:
    sys.path.insert(0, "/opt/trn_rl_repo")

import numpy as np
import ml_dtypes

_bf16 = np.dtype(ml_dtypes.bfloat16)

B, S, H, NH = 4, 2048, 1024, 16
HD = 64
NCORES = 8
F = 512  # features per core (8 heads)
NHEADS = 8  # heads per core
NPAIR = 4  # head pairs per core
HCH = H // 128  # 8 hidden chunks
SCH = S // 128  # 16 sequence chunks
P = 128

_CACHE = {}


def _build_bass():
    import concourse.tile as tile
    from concourse import bacc, mybir
    from contextlib import ExitStack

    f32 = mybir.dt.float32
    f32r = mybir.dt.float32r
    EXP = mybir.ActivationFunctionType.Exp
    ADD = mybir.AluOpType.add
    MULT = mybir.AluOpType.mult

    nc = bacc.Bacc("TRN2", target_bir_lowering=False, debug=False, num_devices=NCORES)

    bf16 = mybir.dt.bfloat16
    x_d = nc.dram_tensor("xb", [S, H], bf16, kind="ExternalInput").ap()
    wq_d = nc.dram_tensor("wqb", [F, H], bf16, kind="ExternalInput").ap()
    wk_d = nc.dram_tensor("wkb", [F, H], bf16, kind="ExternalInput").ap()
    wv_d = nc.dram_tensor("wvb", [F, H], bf16, kind="ExternalInput").ap()
    bqt_d = nc.dram_tensor("bqt", [P, NPAIR], f32, kind="ExternalInput").ap()
    bkt_d = nc.dram_tensor("bkt", [P, NPAIR], f32, kind="ExternalInput").ap()
    bvb_d = nc.dram_tensor("bvb", [P, F], f32, kind="ExternalInput").ap()
    maskb_d = nc.dram_tensor("maskb", [P, SCH], f32, kind="ExternalInput").ap()
    cm_d = nc.dram_tensor("cm", [P, P], bf16, kind="ExternalInput").ap()
    id_d = nc.dram_tensor("ident", [P, P], f32, kind="ExternalInput").ap()
    out_d = nc.dram_tensor("out", [S, F], f32, kind="ExternalOutput").ap()

    with tile.TileContext(nc) as tc, ExitStack() as ctx:
        const = ctx.enter_context(tc.tile_pool(name="const", bufs=1))
        ident = const.tile([P, P], f32, tag="ident")
        nc.sync.dma_start(ident[:], id_d[:])
        cm = const.tile([P, P], bf16, tag="cm")
        nc.sync.dma_start(cm[:], cm_d[:])
        maskb = const.tile([P, SCH], f32, tag="maskb")
        nc.sync.dma_start(maskb[:], maskb_d[:])
        bqt = const.tile([P, NPAIR], f32, tag="bqt")
        nc.sync.dma_start(bqt[:], bqt_d[:])
        bkt = const.tile([P, NPAIR], f32, tag="bkt")
        nc.sync.dma_start(bkt[:], bkt_d[:])
        bvb = const.tile([P, F], f32, tag="bvb")
        nc.sync.dma_start(bvb[:], bvb_d[:])

        xt_pool = ctx.enter_context(tc.tile_pool(name="xt", bufs=1))
        xt = xt_pool.tile([P, HCH, S], bf16, tag="xt")  # X^T via DMA transpose
        v_pool = ctx.enter_context(tc.tile_pool(name="v", bufs=1))
        v = v_pool.tile([P, SCH, NHEADS, HD + 1], bf16, tag="v")  # V + ones col

        # PSUM: QK pair slots (2 x 2 banks), small slots for projections and
        # PE transposes (2 x 1 bank), AV accumulators (2 x 1 bank) = 8 banks.
        mmps = ctx.enter_context(tc.tile_pool(name="mmps", bufs=2, space="PSUM"))
        smps = ctx.enter_context(tc.tile_pool(name="smps", bufs=2, space="PSUM"))
        ops_ = ctx.enter_context(tc.tile_pool(name="ops", bufs=2, space="PSUM"))
        wt_pool = ctx.enter_context(tc.tile_pool(name="wt", bufs=2))
        qkt_pool = ctx.enter_context(tc.tile_pool(name="qkt", bufs=2))
        p_pool = ctx.enter_context(tc.tile_pool(name="pp", bufs=22))
        wtv_pool = ctx.enter_context(tc.tile_pool(name="wtv", bufs=1))
        obuf = ctx.enter_context(tc.tile_pool(name="obuf", bufs=4))
        rec_pool = ctx.enter_context(tc.tile_pool(name="rec", bufs=4))

        # ---- per head-pair: project Q^T/K^T, then attention ----
        # The projection work of pair p+1 is emitted in fine-grained units
        # interleaved into pair p's attention steps, placed between the
        # next QK prefetch and the exp-dependent AV matmuls so the PE has
        # independent work while ScalarE computes exp.
        def make_pair_proj(pr):
            wtq = wt_pool.tile([P, HCH, P], bf16, tag="wtq")
            wtk = wt_pool.tile([P, HCH, P], bf16, tag="wtk")
            qt = qkt_pool.tile([P, S], bf16, tag="qt")
            kt = qkt_pool.tile([P, S], bf16, tag="kt")
            units = []
            for wd, wt in ((wq_d, wtq), (wk_d, wtk)):

                def dma_u(wt=wt, wd=wd):
                    for j in range(HCH):
                        nc.sync.dma_start_transpose(
                            wt[:, j, :],
                            wd[pr * 128 : (pr + 1) * 128, j * 128 : (j + 1) * 128],
                        )

                units.append(dma_u)
            for wt, dst, bias in ((wtq, qt, bqt), (wtk, kt, bkt)):
                for st in range(4):
                    ps = smps.tile([P, F], f32, tag="sm")
                    for j0 in range(0, HCH, 2):

                        def mm_u(wt=wt, ps=ps, st=st, j0=j0):
                            for j in (j0, j0 + 1):
                                nc.tensor.matmul(
                                    ps[:],
                                    wt[:, j, :],
                                    xt[:, j, st * 512 : (st + 1) * 512],
                                    start=(j == 0),
                                    stop=(j == HCH - 1),
                                )

                        units.append(mm_u)

                    def cb_u(dst=dst, ps=ps, st=st, bias=bias):
                        nc.vector.tensor_scalar_add(
                            dst[:, st * 512 : (st + 1) * 512],
                            ps[:],
                            bias[:, pr : pr + 1],
                        )

                    units.append(cb_u)
            return qt, kt, units

        # ---- A0: Wv^T/X^T DMA transposes interleaved with V-projection and
        # pair-0's Q^T/K^T projection, per 512-wide s-block, so the PE has
        # matmul work as soon as the first slices of X^T land. ----
        nc.vector.tensor_scalar(
            v[:, :, :, HD : HD + 1],
            bvb[:, 0:128].rearrange("p (a b c) -> p a b c", a=SCH, b=NHEADS),
            0.0,
            1.0,
            MULT,
            ADD,
        )
        pair_state = {0: make_pair_proj(0)}
        p0u = pair_state[0][2]
        assert len(p0u) == 42
        p0u[0]()
        p0u[1]()
        wtv = wtv_pool.tile([P, HCH, F], bf16, tag="wtv")
        for sb in range(4):
            for j in range(HCH):
                nc.sync.dma_start_transpose(
                    xt[:, j, sb * 512 : (sb + 1) * 512],
                    x_d[sb * 512 : (sb + 1) * 512, j * 128 : (j + 1) * 128],
                )
            if sb == 0:
                for j in range(HCH):
                    nc.sync.dma_start_transpose(
                        wtv[:, j, :], wv_d[:, j * 128 : (j + 1) * 128]
                    )
            for u in p0u[2 + 5 * sb : 7 + 5 * sb]:
                u()
            for u in p0u[22 + 5 * sb : 27 + 5 * sb]:
                u()
            for si in range(4 * sb, 4 * sb + 4):
                ps = mmps.tile([P, 1024], f32, tag="mm")
                for j in range(HCH):
                    nc.tensor.matmul(
                        ps[:, 0:F],
                        xt[:, j, si * 128 : (si + 1) * 128],
                        wtv[:, j, :],
                        start=(j == 0),
                        stop=(j == HCH - 1),
                    )
                nc.vector.tensor_tensor(
                    v[:, si, :, 0:HD],
                    ps[:, 0:F].rearrange("p (h d) -> p h d", h=NHEADS),
                    bvb[:].rearrange("p (h d) -> p h d", h=NHEADS),
                    ADD,
                )

        # ---- attention: QK + exp stream per q-tile; AV runs in natural
        # layout (out[q, d+1] = P^T-chunk.T @ V_aug) as deferred fill units
        # drained between QK steps — full 128-row array utilization and the
        # softmax denominator arrives as column HD of each accumulator. ----
        from collections import deque

        fillq = deque()

        def emit_fill(n):
            while n > 0 and fillq:
                fillq.popleft()()
                n -= 1

        def make_av_unit(pts, qc, h, hb, q0, qi):
            def av_unit():
                nkq = 4 * qi + qc + 1
                o_ps = ops_.tile([P, HD + 1], f32, tag="o")
                for kc in range(nkq):
                    nc.tensor.matmul(
                        o_ps[:],
                        pts[kc][:, hb + qc * 128 : hb + (qc + 1) * 128],
                        v[:, kc, h, :],
                        start=(kc == 0),
                        stop=(kc == nkq - 1),
                    )
                rec = rec_pool.tile([P, 1], f32, tag="rec")
                nc.vector.reciprocal(rec[:], o_ps[:, HD : HD + 1])
                otile = obuf.tile([P, HD], f32, tag="ob")
                nc.vector.tensor_scalar_mul(otile[:], o_ps[:, 0:HD], rec[:])
                nc.sync.dma_start(
                    out_d[q0 + qc * 128 : q0 + (qc + 1) * 128, h * HD : (h + 1) * HD],
                    otile[:],
                )

            return av_unit

        for pr in range(NPAIR):
            qt, kt, _ = pair_state[pr]
            if pr + 1 < NPAIR:
                pair_state[pr + 1] = make_pair_proj(pr + 1)
                fillq.extend(pair_state[pr + 1][2])
            h0, h1 = 2 * pr, 2 * pr + 1

            def emit_qk(qi, kc, qt=qt, kt=kt):
                q0 = qi * 512
                off = kc - 4 * qi
                lo = off * 128 if off > 0 else 0
                ps = mmps.tile([P, 1024], f32, tag="mm")
                nc.tensor.matmul(
                    ps[:, lo:512],
                    kt[0:64, kc * 128 : (kc + 1) * 128],
                    qt[0:64, q0 + lo : q0 + 512],
                    start=True,
                    stop=True,
                )
                nc.tensor.matmul(
                    ps[:, 512 + lo : 1024],
                    kt[64:128, kc * 128 : (kc + 1) * 128],
                    qt[64:128, q0 + lo : q0 + 512],
                    start=True,
                    stop=True,
                )
                return ps

            for qi in range(4):
                q0 = qi * 512
                nk = 4 * (qi + 1)
                pts = []
                ps = emit_qk(qi, 0)
                for kc in range(nk):
                    off = kc - 4 * qi
                    lo = off * 128 if off > 0 else 0
                    pt = p_pool.tile([P, 1024], bf16, tag="pt")
                    pts.append(pt)
                    if lo == 0:
                        nc.scalar.activation(
                            pt[:], ps[:], EXP, bias=maskb[:, kc : kc + 1], scale=0.125
                        )
                    else:
                        nc.scalar.activation(
                            pt[:, lo:512],
                            ps[:, lo:512],
                            EXP,
                            bias=maskb[:, kc : kc + 1],
                            scale=0.125,
                        )
                        nc.scalar.activation(
                            pt[:, 512 + lo : 1024],
                            ps[:, 512 + lo : 1024],
                            EXP,
                            bias=maskb[:, kc : kc + 1],
                            scale=0.125,
                        )
                    if off >= 0:
                        nc.vector.tensor_mul(
                            pt[:, lo : lo + 128], pt[:, lo : lo + 128], cm[:]
                        )
                        nc.vector.tensor_mul(
                            pt[:, 512 + lo : 512 + lo + 128],
                            pt[:, 512 + lo : 512 + lo + 128],
                            cm[:],
                        )
                    if kc + 1 < nk:
                        ps = emit_qk(qi, kc + 1)
                    emit_fill(2)
                for qc in range(4):
                    fillq.append(make_av_unit(pts, qc, h0, 0, q0, qi))
                    fillq.append(make_av_unit(pts, qc, h1, 512, q0, qi))
        emit_fill(10**9)

    nc.compile()
    return nc


def _get_nc():
    if "nc" not in _CACHE:
        _CACHE["nc"] = _build_bass()
    return _CACHE["nc"]


def _host_consts():
    if "consts" not in _CACHE:
        qq = np.arange(P)[None, :]
        kk = np.arange(P)[:, None]
        _CACHE["consts"] = {
            "cm": (qq >= kk).astype(_bf16),
            "ident": np.eye(P, dtype=np.float32),
        }
    return _CACHE["consts"]


def make_in_maps(inputs):
    hs = np.asarray(inputs["hidden_states"], dtype=np.float32)
    am = np.asarray(inputs["attention_mask"], dtype=np.float32)
    Wq = np.asarray(inputs["Wq"], dtype=np.float32)
    bq = np.asarray(inputs["bq"], dtype=np.float32)
    Wk = np.asarray(inputs["Wk"], dtype=np.float32)
    bk = np.asarray(inputs["bk"], dtype=np.float32)
    Wv = np.asarray(inputs["Wv"], dtype=np.float32)
    bv = np.asarray(inputs["bv"], dtype=np.float32)

    consts = _host_consts()
    in_maps = []
    for c in range(NCORES):
        b, hg = c // 2, c % 2
        fsl = slice(hg * F, (hg + 1) * F)
        in_maps.append(
            {
                "xb": np.ascontiguousarray(hs[b]).astype(_bf16),
                "wqb": np.ascontiguousarray(Wq[fsl]).astype(_bf16),
                "wkb": np.ascontiguousarray(Wk[fsl]).astype(_bf16),
                "wvb": np.ascontiguousarray(Wv[fsl]).astype(_bf16),
                "bqt": np.ascontiguousarray(bq[fsl].reshape(NPAIR, P).T),
                "bkt": np.ascontiguousarray(bk[fsl].reshape(NPAIR, P).T),
                "bvb": np.broadcast_to(bv[fsl], (P, F)).copy(),
                "maskb": np.ascontiguousarray((am[b, 0, 0] / 8.0).reshape(SCH, P).T),
                "cm": consts["cm"],
                "ident": consts["ident"],
            }
        )
    return in_maps


def assemble_out(results):
    out = np.empty((B, S, H), dtype=np.float32)
    for c in range(NCORES):
        b, hg = c // 2, c % 2
        out[b, :, hg * F : (hg + 1) * F] = results[c]["out"]
    return out


def kernel(**inputs):
    from concourse.bass_utils import run_bass_kernel_spmd

    in_maps = make_in_maps(inputs)
    nc = _get_nc()
    res = run_bass_kernel_spmd(nc, in_maps, list(range(NCORES)))
    return assemble_out(res.results)


if __name__ == "__main__":
    rng = np.random.default_rng(0)
    ins = {
        "hidden_states": rng.standard_normal((B, S, H)).astype(np.float32),
        "attention_mask": np.zeros((B, 1, 1, S), np.float32),
        "Wq": (rng.standard_normal((H, H)) / 32.0).astype(np.float32),
        "bq": np.zeros(H, np.float32),
        "Wk": (rng.standard_normal((H, H)) / 32.0).astype(np.float32),
        "bk": np.zeros(H, np.float32),
        "Wv": (rng.standard_normal((H, H)) / 32.0).astype(np.float32),
        "bv": np.zeros(H, np.float32),
    }
    o = kernel(**ins)
    print("out", o.shape, o.dtype, float(np.abs(o).max()))
